# revision 33
# baseline (speedup 1.0000x reference)
"""Causal attention kernel for Trainium2 (Bass/Tile), 8-core SPMD.

Problem: x:(4,2048,1024), w_{q,k,v}:(1024,1024) fp32.
  q/k/v = x @ w.T ; scores = (q @ k.T)/sqrt(1024) causal-masked; out = softmax @ v.

Sharding: core c -> batch b=c//2, half h=c%2. Each batch's 16 query blocks
(128 rows) are interleaved even/odd between its two cores (core-local block
j <-> global block g=2j+h), so causal work is balanced. No inter-core
communication at all. The program is identical on all cores (SPMD); per-core
differences are input DATA only (which rows go into xqTb, and the cmask
whose diagonal offset encodes h).

Math restructuring (the win vs a direct QKV kernel): neither Q, K nor V is
ever materialized on-chip.
  scores = x_q (Wq^T Wk) x_k^T : the host precomputes M = Wq^T @ Wk (an input
    transform like the host transposes); per query block the kernel does
    A = x_q @ M (one small GEMM), then scores = A @ x^T where the
    pre-transposed x^T is the key-side operand directly.
  context = softmax @ (x Wv^T) = (softmax @ x) @ Wv^T : accumulate
    U = P^T-chunks @ x (natural layout) over the causal key range, then one
    [1024 x 128q x 1024] GEMM with Wv^T. This replaces the full-T V
    projection (131k PE-cycles) with U (73.7k) + U^T (8k) + U@WvT (65.5k).
This removes the K/Q/V projections AND the K^T/V AllGather pair of a naive
B/2-sharding; per-core PE work is ~296k matmul-cycles, ~62us at the
measured 2-cols/cycle bf16 stream rate, balanced against ~18MB of HBM
traffic (~50us) moved with coarse >=2KB-line DMA descriptors. Measured
body (slope method): ~60us.

Layouts (host pre-transposes/casts; bf16 except cmask):
  m_b   [D, D]    bf16 = Wq^T @ Wk
  xTb   [D, T]    bf16 = x[b].T       (d on partitions: scores rhs)
  x_nb  [T, D]    bf16 = x[b]         (t on partitions: U rhs)
  xqTb  [D, 1024] bf16 = x[b][qrows].T (own query rows, A lhsT)
  wvTb  [D, D]    bf16 = Wv^T         (ctx rhs)
  cmask [128, 256] f32 additive mask for the last two key blocks of a qblock
Phases per core: A/A^T for the 8 query blocks (PE transposes, cast bf16) ->
per query block smallest-first (DMA streams ahead of compute): scores into
PSUM slices (PE) -> mask add (DVE) -> exp+row-sum straight from PSUM (ACT,
fused accum_out, bf16 out) -> P^T via PE transpose (bf16) -> U accumulation
(PE) -> then, pipelined one block behind: U^T via PE transpose -> context
GEMM vs Wv^T -> scale by 1/rowsum (DVE) -> DMA out.
Measured rel err ~4e-3 (tolerance 2e-2).
"""

import numpy as np

_B, _T, _D = 4, 2048, 1024
_P = 128
_NQB = 8              # query blocks per core
_TQ = _NQB * _P       # 1024 query rows per core
_GAMMA = 1.0 / 32.0   # 1/sqrt(D)
_NEG = -1.0e9

_MM_MODE = "fp32r"    # kept for test.py compat; scores/ctx path is bf16
_SUB_MAX = False      # scores ~N(0,1) after scaling; exp can't overflow

_CACHE = {}


def _build_nc(mm_mode: str = "fp32r", sub_max: bool = False, reps: int = 1):
    import concourse.bass as bass  # noqa: F401
    import concourse.mybir as mybir
    import concourse.tile as tile
    from concourse import bacc
    from concourse.masks import make_identity
    from contextlib import ExitStack

    f32 = mybir.dt.float32
    bf = mybir.dt.bfloat16

    nc = bacc.Bacc(None, target_bir_lowering=False)
    m_b = nc.dram_tensor("m_b", [_D, _D], bf, kind="ExternalInput")
    xTb = nc.dram_tensor("xTb", [_D, _T], bf, kind="ExternalInput")
    x_nb = nc.dram_tensor("x_nb", [_T, _D], bf, kind="ExternalInput")
    xqTb = nc.dram_tensor("xqTb", [_D, _TQ], bf, kind="ExternalInput")
    wvTb = nc.dram_tensor("wvTb", [_D, _D], bf, kind="ExternalInput")
    cmask = nc.dram_tensor("cmask", [_P, 2 * _P], f32, kind="ExternalInput")
    out = nc.dram_tensor("out", [_TQ, _D], bf, kind="ExternalOutput")

    m_v = m_b.rearrange("(a p) o -> p a o", p=_P)     # [128, 8, 1024] d1->d2
    xTb_v = xTb.rearrange("(a p) t -> p a t", p=_P)   # [128, 8, 2048]
    xn_v = x_nb.rearrange("(tb p) d -> p tb d", p=_P)  # [128, 16, 1024]
    xqT_v = xqTb.rearrange("(a p) q -> p a q", p=_P)  # [128, 8, 1024]
    wvT_v = wvTb.rearrange("(a p) o -> p a o", p=_P)  # [128, 8, 1024]

    with ExitStack() as ctx:
        tc = ctx.enter_context(tile.TileContext(nc))
        const = ctx.enter_context(tc.tile_pool(name="const", bufs=1))

        ident_f = const.tile([_P, _P], f32, tag="identf")
        make_identity(nc, ident_f)
        ident_b = const.tile([_P, _P], bf, tag="identb")
        nc.vector.tensor_copy(ident_b, ident_f)
        cmask_sb = const.tile([_P, 2 * _P], f32, tag="cmask")
        nc.sync.dma_start(out=cmask_sb, in_=cmask[:, :])

        # long-lived SBUF
        pers = ctx.enter_context(tc.tile_pool(name="pers", bufs=1))
        xb_sb = pers.tile([_P, 8, _T], bf, tag="xb")     # scores rhs
        xn_sb = pers.tile([_P, 16, _D], bf, tag="xn")    # U rhs
        AT_sb = pers.tile([_P, 8, _TQ], bf, tag="at")
        wv_sb = pers.tile([_P, 8, _D], bf, tag="wv")

        def _phase_a(asc):
            # ---- A^T = M^T @ x_q directly: out[d2, q] = sum_d1 M[d1, d2]
            # xq^T[d1, q]; lhsT = M chunks (natural layout), rhs = xq^T.
            # No PE transposes needed at all for the scores lhsT.
            with tc.tile_pool(name="pa_in", bufs=1) as pin, \
                 tc.tile_pool(name="pa_ps", bufs=4, space="PSUM") as paps:
                m_sb = pin.tile([_P, 8, _D], bf, tag="m")
                xq_sb = pin.tile([_P, 8, _TQ], bf, tag="xq")
                # DMA emission order == consumption order: m+xq (A), wv,
                # then the x^T / x-natural key-range halves in the order
                # this rep's phase C touches them (direction alternates per
                # rep so the cross-rep refill hides under compute). Coarse
                # multi-dim descriptors (>=2KB per partition line) keep the
                # DMA engines at full rate.
                nc.sync.dma_start(out=m_sb[:, :, :], in_=m_v[:, :, :])
                nc.sync.dma_start(out=xq_sb[:, :, :], in_=xqT_v[:, :, :])
                nc.sync.dma_start(out=wv_sb[:, :, :], in_=wvT_v[:, :, :])
                for c in ((0, 1) if asc else (1, 0)):
                    nc.sync.dma_start(
                        out=xb_sb[:, :, c * 1024:(c + 1) * 1024],
                        in_=xTb_v[:, :, c * 1024:(c + 1) * 1024])
                    nc.sync.dma_start(out=xn_sb[:, 8 * c:8 * c + 8, :],
                                      in_=xn_v[:, 8 * c:8 * c + 8, :])

                for ns in range(2):
                    for a2 in range(8):
                        at_ps = paps.tile([_P, 512], f32, tag="aps")
                        for dc in range(8):
                            nc.tensor.matmul(
                                at_ps,
                                m_sb[:, dc, a2 * _P:(a2 + 1) * _P],
                                xq_sb[:, dc, ns * 512:(ns + 1) * 512],
                                start=(dc == 0), stop=(dc == 7))
                        if a2 % 2 == 0:
                            nc.scalar.copy(
                                AT_sb[:, a2, ns * 512:(ns + 1) * 512], at_ps)
                        else:
                            nc.vector.tensor_copy(
                                AT_sb[:, a2, ns * 512:(ns + 1) * 512], at_ps)

        def _phase_c(order):
            # ---- attention per query block in `order` (DMA overlap);
            # finish stage (U^T, ctx GEMM, scale, out) pipelined one block
            # behind so the U->bf16 cast copy hides under the next block's
            # scores matmuls.
            with tc.tile_pool(name="pc_pex", bufs=2) as ppsb, \
                 tc.tile_pool(name="pc_pt", bufs=4) as ppt, \
                 tc.tile_pool(name="pc_u", bufs=2) as pu, \
                 tc.tile_pool(name="pc_ctx", bufs=2) as pctx, \
                 tc.tile_pool(name="pc_small", bufs=8) as psm, \
                 tc.tile_pool(name="pc_ps_s", bufs=2, space="PSUM") as pps, \
                 tc.tile_pool(name="pc_ps_t", bufs=2, space="PSUM") as ppts, \
                 tc.tile_pool(name="pc_ps_u", bufs=1, space="PSUM") as ppu, \
                 tc.tile_pool(name="pc_ps_c", bufs=1, space="PSUM") as ppc:

                def finish(j, U_sb, rden):
                    # U^T via PE transposes interleaved with the ctx GEMMs
                    # so each GEMM pair hides the next transpose's LDWEIGHTS
                    ut = []

                    def emit_tr(dc):
                        ut_ps = ppts.tile([_P, _P], bf, tag="pt")
                        nc.tensor.transpose(
                            ut_ps, U_sb[:, dc * _P:(dc + 1) * _P], ident_b)
                        ut_sb = ppt.tile([_P, _P], bf, tag="pts")
                        nc.vector.tensor_copy(ut_sb, ut_ps)
                        ut.append(ut_sb)

                    emit_tr(0)
                    emit_tr(1)
                    ctx_ps = ppc.tile([_P, _D], f32, tag="ctx")
                    for dc in range(8):
                        if dc + 2 < 8:
                            emit_tr(dc + 2)
                        for ns in range(2):
                            nc.tensor.matmul(
                                ctx_ps[:, ns * 512:(ns + 1) * 512],
                                ut[dc], wv_sb[:, dc, ns * 512:(ns + 1) * 512],
                                start=(dc == 0), stop=(dc == 7))
                    ctx_sb = pctx.tile([_P, _D], bf, tag="ctxsb")
                    nc.vector.tensor_scalar_mul(ctx_sb, ctx_ps, rden)
                    nc.sync.dma_start(
                        out=out[j * _P:(j + 1) * _P, :], in_=ctx_sb)

                prev = None
                for j in order:
                    km = 256 * (j + 1)
                    nkb = 2 * (j + 1)
                    nsl = (km + 511) // 512
                    pexp = ppsb.tile([_P, _T], bf, tag="pexp")
                    denoms = psm.tile([_P, 4], f32, tag="denoms")
                    for ks in range(nsl):
                        w = min(512, km - ks * 512)
                        ps = pps.tile([_P, 512], f32, tag="s")
                        for a2 in range(8):
                            nc.tensor.matmul(
                                ps[:, :w],
                                AT_sb[:, a2, j * _P:(j + 1) * _P],
                                xb_sb[:, a2, ks * 512:ks * 512 + w],
                                start=(a2 == 0), stop=(a2 == 7))
                        if ks == nsl - 1:
                            nc.vector.tensor_add(
                                ps[:, w - 256:w], ps[:, w - 256:w], cmask_sb)
                        nc.scalar.activation(
                            out=pexp[:, ks * 512:ks * 512 + w], in_=ps[:, :w],
                            func=mybir.ActivationFunctionType.Exp,
                            bias=0.0, scale=_GAMMA,
                            accum_out=denoms[:, ks:ks + 1])

                    denom = psm.tile([_P, 1], f32, tag="denom")
                    nc.vector.tensor_reduce(
                        out=denom, in_=denoms[:, :nsl],
                        axis=mybir.AxisListType.X, op=mybir.AluOpType.add)
                    rden = psm.tile([_P, 1], f32, tag="rden")
                    nc.vector.reciprocal(rden, denom)

                    # U = sum_kb P^T(kb) @ x_n(kb); transposes pipelined one
                    # ahead of the U matmuls so the DVE pt copy is hidden.
                    U_ps = ppu.tile([_P, _D], f32, tag="u")
                    pts = []
                    for kb in range(min(2, nkb)):
                        pt_ps = ppts.tile([_P, _P], bf, tag="pt")
                        nc.tensor.transpose(
                            pt_ps, pexp[:, kb * _P:(kb + 1) * _P], ident_b)
                        pt_sb = ppt.tile([_P, _P], bf, tag="pts")
                        nc.vector.tensor_copy(pt_sb, pt_ps)
                        pts.append(pt_sb)
                    for kb in range(nkb):
                        if kb + 2 < nkb:
                            pt_ps = ppts.tile([_P, _P], bf, tag="pt")
                            nc.tensor.transpose(
                                pt_ps,
                                pexp[:, (kb + 2) * _P:(kb + 3) * _P], ident_b)
                            pt_sb = ppt.tile([_P, _P], bf, tag="pts")
                            nc.vector.tensor_copy(pt_sb, pt_ps)
                            pts.append(pt_sb)
                        for ns in range(2):
                            nc.tensor.matmul(
                                U_ps[:, ns * 512:(ns + 1) * 512],
                                pts[kb], xn_sb[:, kb, ns * 512:(ns + 1) * 512],
                                start=(kb == 0), stop=(kb == nkb - 1))
                    U_sb = pu.tile([_P, _D], bf, tag="usb")
                    nc.scalar.copy(U_sb, U_ps)

                    if prev is not None:
                        finish(*prev)
                    prev = (j, U_sb, rden)
                finish(*prev)

        for _rep in range(reps):
            asc = _rep % 2 == 0
            order = list(range(_NQB)) if asc else list(reversed(range(_NQB)))
            _phase_a(asc)
            _phase_c(order)

    nc.finalize()
    return nc


def _qrows(h: int) -> np.ndarray:
    """Global query-row indices handled by half h, in core-local order."""
    blocks = np.arange(_NQB) * 2 + h          # global block ids, 8 of them
    return (blocks[:, None] * _P + np.arange(_P)[None, :]).reshape(-1)


def _host_inputs(x, w_query, w_key, w_value, mm_mode: str = "fp32r"):
    import ml_dtypes
    bf = ml_dtypes.bfloat16
    wq = np.asarray(w_query, np.float32)
    wk = np.asarray(w_key, np.float32)
    wv = np.asarray(w_value, np.float32)
    x = np.asarray(x, np.float32)

    m_b = np.ascontiguousarray(wq.T @ wk).astype(bf)
    wvTb = np.ascontiguousarray(wv.T).astype(bf)

    # shared per-batch / per-half arrays (two cores share a batch)
    xb_bf = [x[b].astype(bf) for b in range(_B)]
    xT_by_b = [np.ascontiguousarray(x[b].T).astype(bf) for b in range(_B)]
    cmask_by_h = []
    p = np.arange(_P)[:, None]
    c2 = np.arange(2 * _P)[None, :]
    for h in range(2):
        cmask_by_h.append(
            np.where(c2 <= p + _P * h, 0.0, _NEG).astype(np.float32))

    in_maps = []
    for c in range(8):
        b, h = c // 2, c % 2
        xqTb = np.ascontiguousarray(x[b][_qrows(h)].T).astype(bf)
        in_maps.append({
            "m_b": m_b, "xTb": xT_by_b[b], "x_nb": xb_bf[b], "xqTb": xqTb,
            "wvTb": wvTb, "cmask": cmask_by_h[h],
        })
    return in_maps


def _gather(results):
    out = np.empty((_B, _T, _D), np.float32)
    for c in range(8):
        b, h = c // 2, c % 2
        out[b, _qrows(h)] = results[c]["out"].astype(np.float32)
    return out


def kernel(x, w_query, w_key, w_value, _trace=False):
    key = (_MM_MODE, _SUB_MAX)
    if key not in _CACHE:
        _CACHE[key] = _build_nc(_MM_MODE, _SUB_MAX)
    nc = _CACHE[key]
    in_maps = _host_inputs(x, w_query, w_key, w_value, _MM_MODE)
    from concourse.bass_utils import run_bass_kernel_spmd
    res = run_bass_kernel_spmd(nc, in_maps, core_ids=list(range(8)),
                               trace=_trace)
    out = _gather(res.results)
    if _trace:
        return out, res
    return out


# revision 35
# speedup vs baseline: 1.8105x; 1.8105x over previous
"""Causal attention kernel for Trainium2 (Bass/Tile), 8-core SPMD.

Problem: x:(4,2048,1024), w_{q,k,v}:(1024,1024) fp32.
  q/k/v = x @ w.T ; scores = (q @ k.T)/sqrt(1024) causal-masked; out = softmax @ v.

Sharding: core c -> batch b=c//2, half h=c%2. Each batch's 16 query blocks
(128 rows) are interleaved even/odd between its two cores (core-local block
j <-> global block g=2j+h), so causal work is balanced. No inter-core
communication at all. The program is identical on all cores (SPMD); per-core
differences are input DATA only (which rows go into xqTb, and the cmask
whose diagonal offset encodes h).

Math restructuring (the win vs a direct QKV kernel): neither Q, K nor V is
ever materialized on-chip.
  scores = x_q (Wq^T Wk) x_k^T : the host precomputes M = Wq^T @ Wk (an input
    transform like the host transposes); per query block the kernel does
    A = x_q @ M (one small GEMM), then scores = A @ x^T where the
    pre-transposed x^T is the key-side operand directly.
  context = softmax @ (x Wv^T) = (softmax @ x) @ Wv^T : accumulate
    U = P^T-chunks @ x (natural layout) over the causal key range, then one
    [1024 x 128q x 1024] GEMM with Wv^T. This replaces the full-T V
    projection (131k PE-cycles) with U (73.7k) + U^T (8k) + U@WvT (65.5k).
This removes the K/Q/V projections AND the K^T/V AllGather pair of a naive
B/2-sharding; per-core PE work is ~296k matmul-cycles, ~62us at the
measured 2-cols/cycle bf16 stream rate, balanced against ~18MB of HBM
traffic (~50us) moved with coarse >=2KB-line DMA descriptors. Measured
body (slope method): ~60us.

Layouts (host pre-transposes/casts; bf16 except cmask):
  m_b   [D, D]    bf16 = Wq^T @ Wk
  xTb   [D, T]    bf16 = x[b].T       (d on partitions: scores rhs)
  x_nb  [T, D]    bf16 = x[b]         (t on partitions: U rhs)
  xqTb  [D, 1024] bf16 = x[b][qrows].T (own query rows, A lhsT)
  wvTb  [D, D]    bf16 = Wv^T         (ctx rhs)
  cmask [128, 256] f32 additive mask for the last two key blocks of a qblock
Phases per core: A/A^T for the 8 query blocks (PE transposes, cast bf16) ->
per query block smallest-first (DMA streams ahead of compute): scores into
PSUM slices (PE) -> mask add (DVE) -> exp+row-sum straight from PSUM (ACT,
fused accum_out, bf16 out) -> P^T via PE transpose (bf16) -> U accumulation
(PE) -> then, pipelined one block behind: U^T via PE transpose -> context
GEMM vs Wv^T -> scale by 1/rowsum (DVE) -> DMA out.
Measured rel err ~4e-3 (tolerance 2e-2).
"""

import numpy as np

_B, _T, _D = 4, 2048, 1024
_P = 128
_NQB = 8              # query blocks per core
_TQ = _NQB * _P       # 1024 query rows per core
_GAMMA = 1.0 / 32.0   # 1/sqrt(D)
_NEG = -1.0e9

_MM_MODE = "fp32r"    # kept for test.py compat; scores/ctx path is bf16
_SUB_MAX = False      # scores ~N(0,1) after scaling; exp can't overflow

_CACHE = {}


def _build_nc(mm_mode: str = "fp32r", sub_max: bool = False, reps: int = 1):
    import concourse.bass as bass  # noqa: F401
    import concourse.mybir as mybir
    import concourse.tile as tile
    from concourse import bacc
    from concourse.masks import make_identity
    from contextlib import ExitStack

    f32 = mybir.dt.float32
    bf = mybir.dt.bfloat16

    nc = bacc.Bacc(None, target_bir_lowering=False)
    m_b = nc.dram_tensor("m_b", [_D, _D], bf, kind="ExternalInput")
    xTb = nc.dram_tensor("xTb", [_D, _T], bf, kind="ExternalInput")
    x_nb = nc.dram_tensor("x_nb", [_T, _D], bf, kind="ExternalInput")
    xqTb = nc.dram_tensor("xqTb", [_D, _TQ], bf, kind="ExternalInput")
    wvTb = nc.dram_tensor("wvTb", [_D, _D], bf, kind="ExternalInput")
    cmask = nc.dram_tensor("cmask", [_P, 2 * _P], f32, kind="ExternalInput")
    out = nc.dram_tensor("out", [_TQ, _D], bf, kind="ExternalOutput")

    m_v = m_b.rearrange("(a p) o -> p a o", p=_P)     # [128, 8, 1024] d1->d2
    xTb_v = xTb.rearrange("(a p) t -> p a t", p=_P)   # [128, 8, 2048]
    xn_v = x_nb.rearrange("(tb p) d -> p tb d", p=_P)  # [128, 16, 1024]
    xqT_v = xqTb.rearrange("(a p) q -> p a q", p=_P)  # [128, 8, 1024]
    wvT_v = wvTb.rearrange("(a p) o -> p a o", p=_P)  # [128, 8, 1024]

    with ExitStack() as ctx:
        tc = ctx.enter_context(tile.TileContext(nc))
        const = ctx.enter_context(tc.tile_pool(name="const", bufs=1))

        ident_f = const.tile([_P, _P], f32, tag="identf")
        make_identity(nc, ident_f)
        ident_b = const.tile([_P, _P], bf, tag="identb")
        nc.vector.tensor_copy(ident_b, ident_f)
        cmask_sb = const.tile([_P, 2 * _P], f32, tag="cmask")
        nc.sync.dma_start(out=cmask_sb, in_=cmask[:, :])

        # long-lived SBUF
        pers = ctx.enter_context(tc.tile_pool(name="pers", bufs=1))
        xb_sb = pers.tile([_P, 8, _T], bf, tag="xb")     # scores rhs
        xn_sb = pers.tile([_P, 16, _D], bf, tag="xn")    # U rhs
        AT_sb = pers.tile([_P, 8, _TQ], bf, tag="at")
        wv_sb = pers.tile([_P, 8, _D], bf, tag="wv")

        def _phase_a(asc):
            # ---- A^T = M^T @ x_q directly: out[d2, q] = sum_d1 M[d1, d2]
            # xq^T[d1, q]; lhsT = M chunks (natural layout), rhs = xq^T.
            # No PE transposes needed at all for the scores lhsT.
            with tc.tile_pool(name="pa_in", bufs=1) as pin, \
                 tc.tile_pool(name="pa_ps", bufs=4, space="PSUM") as paps:
                m_sb = pin.tile([_P, 8, _D], bf, tag="m")
                xq_sb = pin.tile([_P, 8, _TQ], bf, tag="xq")
                # DMA emission order == consumption order: m+xq (A), wv,
                # then the x^T / x-natural key-range halves in the order
                # this rep's phase C touches them (direction alternates per
                # rep so the cross-rep refill hides under compute). Coarse
                # multi-dim descriptors (>=2KB per partition line) keep the
                # DMA engines at full rate.
                nc.sync.dma_start(out=m_sb[:, :, :], in_=m_v[:, :, :])
                nc.sync.dma_start(out=xq_sb[:, :, :], in_=xqT_v[:, :, :])
                nc.sync.dma_start(out=wv_sb[:, :, :], in_=wvT_v[:, :, :])
                for c in ((0, 1) if asc else (1, 0)):
                    nc.sync.dma_start(
                        out=xb_sb[:, :, c * 1024:(c + 1) * 1024],
                        in_=xTb_v[:, :, c * 1024:(c + 1) * 1024])
                    nc.sync.dma_start(out=xn_sb[:, 8 * c:8 * c + 8, :],
                                      in_=xn_v[:, 8 * c:8 * c + 8, :])

                for ns in range(2):
                    for a2 in range(8):
                        at_ps = paps.tile([_P, 512], f32, tag="aps")
                        for dc in range(8):
                            nc.tensor.matmul(
                                at_ps,
                                m_sb[:, dc, a2 * _P:(a2 + 1) * _P],
                                xq_sb[:, dc, ns * 512:(ns + 1) * 512],
                                start=(dc == 0), stop=(dc == 7))
                        if a2 % 2 == 0:
                            nc.scalar.copy(
                                AT_sb[:, a2, ns * 512:(ns + 1) * 512], at_ps)
                        else:
                            nc.vector.tensor_copy(
                                AT_sb[:, a2, ns * 512:(ns + 1) * 512], at_ps)

        def _phase_c(order):
            # ---- attention per query block in `order` (DMA overlap);
            # finish stage (U^T, ctx GEMM, scale, out) pipelined one block
            # behind so the U->bf16 cast copy hides under the next block's
            # scores matmuls.
            with tc.tile_pool(name="pc_pex", bufs=2) as ppsb, \
                 tc.tile_pool(name="pc_pt", bufs=4) as ppt, \
                 tc.tile_pool(name="pc_u", bufs=2) as pu, \
                 tc.tile_pool(name="pc_ctx", bufs=2) as pctx, \
                 tc.tile_pool(name="pc_small", bufs=8) as psm, \
                 tc.tile_pool(name="pc_ps_s", bufs=2, space="PSUM") as pps, \
                 tc.tile_pool(name="pc_ps_t", bufs=2, space="PSUM") as ppts, \
                 tc.tile_pool(name="pc_ps_u", bufs=1, space="PSUM") as ppu, \
                 tc.tile_pool(name="pc_ps_c", bufs=1, space="PSUM") as ppc:

                def finish(j, U_sb, rden):
                    # U^T via PE transposes interleaved with the ctx GEMMs
                    # so each GEMM pair hides the next transpose's LDWEIGHTS
                    ut = []

                    def emit_tr(dc):
                        ut_ps = ppts.tile([_P, _P], bf, tag="pt")
                        nc.tensor.transpose(
                            ut_ps, U_sb[:, dc * _P:(dc + 1) * _P], ident_b)
                        ut_sb = ppt.tile([_P, _P], bf, tag="pts")
                        nc.vector.tensor_copy(ut_sb, ut_ps)
                        ut.append(ut_sb)

                    emit_tr(0)
                    emit_tr(1)
                    ctx_ps = ppc.tile([_P, _D], f32, tag="ctx")
                    for dc in range(8):
                        if dc + 2 < 8:
                            emit_tr(dc + 2)
                        for ns in range(2):
                            nc.tensor.matmul(
                                ctx_ps[:, ns * 512:(ns + 1) * 512],
                                ut[dc], wv_sb[:, dc, ns * 512:(ns + 1) * 512],
                                start=(dc == 0), stop=(dc == 7))
                    ctx_sb = pctx.tile([_P, _D], bf, tag="ctxsb")
                    nc.vector.tensor_scalar_mul(ctx_sb, ctx_ps, rden)
                    nc.sync.dma_start(
                        out=out[j * _P:(j + 1) * _P, :], in_=ctx_sb)

                prev = None
                for j in order:
                    km = 256 * (j + 1)
                    nkb = 2 * (j + 1)
                    nsl = (km + 511) // 512
                    pexp = ppsb.tile([_P, _T], bf, tag="pexp")
                    denoms = psm.tile([_P, 4], f32, tag="denoms")
                    for ks in range(nsl):
                        w = min(512, km - ks * 512)
                        ps = pps.tile([_P, 512], f32, tag="s")
                        for a2 in range(8):
                            nc.tensor.matmul(
                                ps[:, :w],
                                AT_sb[:, a2, j * _P:(j + 1) * _P],
                                xb_sb[:, a2, ks * 512:ks * 512 + w],
                                start=(a2 == 0), stop=(a2 == 7))
                        if ks == nsl - 1:
                            nc.vector.tensor_add(
                                ps[:, w - 256:w], ps[:, w - 256:w], cmask_sb)
                        nc.scalar.activation(
                            out=pexp[:, ks * 512:ks * 512 + w], in_=ps[:, :w],
                            func=mybir.ActivationFunctionType.Exp,
                            bias=0.0, scale=_GAMMA,
                            accum_out=denoms[:, ks:ks + 1])

                    denom = psm.tile([_P, 1], f32, tag="denom")
                    nc.vector.tensor_reduce(
                        out=denom, in_=denoms[:, :nsl],
                        axis=mybir.AxisListType.X, op=mybir.AluOpType.add)
                    rden = psm.tile([_P, 1], f32, tag="rden")
                    nc.vector.reciprocal(rden, denom)

                    # U = sum_kb P^T(kb) @ x_n(kb); transposes pipelined one
                    # ahead of the U matmuls so the DVE pt copy is hidden.
                    U_ps = ppu.tile([_P, _D], f32, tag="u")
                    pts = []
                    for kb in range(min(2, nkb)):
                        pt_ps = ppts.tile([_P, _P], bf, tag="pt")
                        nc.tensor.transpose(
                            pt_ps, pexp[:, kb * _P:(kb + 1) * _P], ident_b)
                        pt_sb = ppt.tile([_P, _P], bf, tag="pts")
                        nc.vector.tensor_copy(pt_sb, pt_ps)
                        pts.append(pt_sb)
                    for kb in range(nkb):
                        if kb + 2 < nkb:
                            pt_ps = ppts.tile([_P, _P], bf, tag="pt")
                            nc.tensor.transpose(
                                pt_ps,
                                pexp[:, (kb + 2) * _P:(kb + 3) * _P], ident_b)
                            pt_sb = ppt.tile([_P, _P], bf, tag="pts")
                            nc.vector.tensor_copy(pt_sb, pt_ps)
                            pts.append(pt_sb)
                        for ns in range(2):
                            nc.tensor.matmul(
                                U_ps[:, ns * 512:(ns + 1) * 512],
                                pts[kb], xn_sb[:, kb, ns * 512:(ns + 1) * 512],
                                start=(kb == 0), stop=(kb == nkb - 1))
                    U_sb = pu.tile([_P, _D], bf, tag="usb")
                    nc.scalar.copy(U_sb, U_ps)

                    if prev is not None:
                        finish(*prev)
                    prev = (j, U_sb, rden)
                finish(*prev)

        for _rep in range(reps):
            asc = _rep % 2 == 0
            order = list(range(_NQB)) if asc else list(reversed(range(_NQB)))
            _phase_a(asc)
            _phase_c(order)

    nc.finalize()
    return nc


def _qrows(h: int) -> np.ndarray:
    """Global query-row indices handled by half h, in core-local order."""
    blocks = np.arange(_NQB) * 2 + h          # global block ids, 8 of them
    return (blocks[:, None] * _P + np.arange(_P)[None, :]).reshape(-1)


def _host_inputs(x, w_query, w_key, w_value, mm_mode: str = "fp32r"):
    import ml_dtypes
    bf = ml_dtypes.bfloat16
    wq = np.asarray(w_query, np.float32)
    wk = np.asarray(w_key, np.float32)
    wv = np.asarray(w_value, np.float32)
    x = np.asarray(x, np.float32)

    m_b = np.ascontiguousarray(wq.T @ wk).astype(bf)
    wvTb = np.ascontiguousarray(wv.T).astype(bf)

    # shared per-batch / per-half arrays (two cores share a batch)
    xb_bf = [x[b].astype(bf) for b in range(_B)]
    xT_by_b = [np.ascontiguousarray(x[b].T).astype(bf) for b in range(_B)]
    cmask_by_h = []
    p = np.arange(_P)[:, None]
    c2 = np.arange(2 * _P)[None, :]
    for h in range(2):
        cmask_by_h.append(
            np.where(c2 <= p + _P * h, 0.0, _NEG).astype(np.float32))

    in_maps = []
    for c in range(8):
        b, h = c // 2, c % 2
        xqTb = np.ascontiguousarray(x[b][_qrows(h)].T).astype(bf)
        in_maps.append({
            "m_b": m_b, "xTb": xT_by_b[b], "x_nb": xb_bf[b], "xqTb": xqTb,
            "wvTb": wvTb, "cmask": cmask_by_h[h],
        })
    return in_maps


def _gather(results):
    out = np.empty((_B, _T, _D), np.float32)
    for c in range(8):
        b, h = c // 2, c % 2
        out[b, _qrows(h)] = results[c]["out"].astype(np.float32)
    return out


def kernel(x, w_query, w_key, w_value, _trace=False):
    key = (_MM_MODE, _SUB_MAX)
    if key not in _CACHE:
        _CACHE[key] = _build_nc(_MM_MODE, _SUB_MAX)
    nc = _CACHE[key]
    in_maps = _host_inputs(x, w_query, w_key, w_value, _MM_MODE)
    from concourse.bass_utils import run_bass_kernel_spmd
    res = run_bass_kernel_spmd(nc, in_maps, core_ids=list(range(8)),
                               trace=_trace)
    out = _gather(res.results)
    if _trace:
        return out, res
    return out
